# revision 7
# baseline (speedup 1.0000x reference)
"""ETNN messager layer on 8 Trainium2 NeuronCores.

Edge-parallel, receiver-sharded: host sorts edges by receiver; core k owns
receivers [k*12500,(k+1)*12500) and scatter-adds into its private slice.
Gathers/scatter use indirect_dma_start ([P,1] per-partition offsets, int32).
BN folded into W1 on host. Messages: silu(state @ W1f + b1f),
gate = sigmoid(msg @ W2 + b2). Receivers within a chunk are made distinct by
column-major spreading so CCE-add scatters never collide inside one
instruction; pads go to a dump row.
"""

import numpy as np

import concourse.tile as tile
from concourse import bacc, bass, mybir
from concourse.bass_utils import run_bass_kernel_spmd
from concourse.masks import make_identity

N = 100000
E = 500000
H = 128
INV = 16
NCORES = 8
NLOC = N // NCORES          # 12500 receivers per core
CHUNK = 2048
NCHUNK = 36
SLOTS = NCHUNK * CHUNK      # 73728 slots/core
ST = CHUNK // 128           # 16 subtiles per chunk
BN_EPS = 1e-5

_prog_cache = {}


def _build(b2val: float):
    key = round(b2val, 9)
    if key in _prog_cache:
        return _prog_cache[key]
    nc = bacc.Bacc("TRN2", target_bir_lowering=False, debug=False)
    dt = mybir.dt
    xs = nc.dram_tensor("xs", [N, H], dt.float32, kind="ExternalInput")
    xr = nc.dram_tensor("xr", [NLOC + 1, H], dt.float32, kind="ExternalInput")
    sidx = nc.dram_tensor("sidx", [128, SLOTS // 128], dt.int32, kind="ExternalInput")
    ridx = nc.dram_tensor("ridx", [128, SLOTS // 128], dt.int32, kind="ExternalInput")
    eat = nc.dram_tensor("eat", [INV + 1, SLOTS], dt.float32, kind="ExternalInput")
    wa = nc.dram_tensor("wa", [H, H], dt.float32, kind="ExternalInput")
    wb = nc.dram_tensor("wb", [H, H], dt.float32, kind="ExternalInput")
    wc = nc.dram_tensor("wc", [INV + 1, H], dt.float32, kind="ExternalInput")
    w2b = nc.dram_tensor("w2b", [128, H], dt.float32, kind="ExternalInput")
    out = nc.dram_tensor("out", [NLOC + 1, H], dt.float32, kind="ExternalOutput")

    with tile.TileContext(nc) as tc:
        with tc.tile_pool(name="const", bufs=1) as cp, \
             tc.tile_pool(name="gath", bufs=4) as gp, \
             tc.tile_pool(name="trans", bufs=4) as tp, \
             tc.tile_pool(name="ea", bufs=3) as ep, \
             tc.tile_pool(name="msg", bufs=2) as mp, \
             tc.tile_pool(name="small", bufs=4) as sp, \
             tc.tile_pool(name="psum", bufs=2, space="PSUM") as pp:
            wa_sb = cp.tile([H, H], dt.float32)
            wb_sb = cp.tile([H, H], dt.float32)
            wc_sb = cp.tile([INV + 1, H], dt.float32)
            w2_sb = cp.tile([128, H], dt.float32)
            si_sb = cp.tile([128, SLOTS // 128], dt.int32)
            ri_sb = cp.tile([128, SLOTS // 128], dt.int32)
            ident = cp.tile([128, 128], dt.float32)
            make_identity(nc, ident[:])
            nc.sync.dma_start(out=wa_sb[:], in_=wa[:, :])
            nc.sync.dma_start(out=wb_sb[:], in_=wb[:, :])
            nc.sync.dma_start(out=wc_sb[:], in_=wc[:, :])
            nc.sync.dma_start(out=w2_sb[:], in_=w2b[:, :])
            nc.sync.dma_start(out=si_sb[:], in_=sidx[:, :])
            nc.sync.dma_start(out=ri_sb[:], in_=ridx[:, :])

            for cl in range(NCHUNK):
                ea_sb = ep.tile([INV + 1, CHUNK], dt.float32, tag="ea")
                nc.sync.dma_start(
                    out=ea_sb[:], in_=eat[:, cl * CHUNK : (cl + 1) * CHUNK]
                )
                msg = mp.tile([128, ST, H], dt.float32, tag="m")
                tt = mp.tile([128, ST, H], dt.float32, tag="t")
                ff = mp.tile([128, ST, H], dt.float32, tag="f")
                red = sp.tile([128, ST], dt.float32, tag="red")
                gate = sp.tile([128, ST], dt.float32, tag="gate")
                for j in range(ST):
                    q0 = cl * ST + j  # subtile column in idx tensors
                    js = slice(j * 128, (j + 1) * 128)
                    gs = gp.tile([128, H], dt.float32, tag="gs")
                    gr = gp.tile([128, H], dt.float32, tag="gr")
                    nc.gpsimd.indirect_dma_start(
                        out=gs[:], out_offset=None, in_=xs[:, :],
                        in_offset=bass.IndirectOffsetOnAxis(
                            ap=si_sb[:, q0 : q0 + 1], axis=0),
                    )
                    nc.gpsimd.indirect_dma_start(
                        out=gr[:], out_offset=None, in_=xr[:, :],
                        in_offset=bass.IndirectOffsetOnAxis(
                            ap=ri_sb[:, q0 : q0 + 1], axis=0),
                    )
                    tps = pp.tile([128, H], dt.float32, tag="tps")
                    tpr = pp.tile([128, H], dt.float32, tag="tpr")
                    nc.tensor.transpose(out=tps[:], in_=gs[:], identity=ident[:])
                    nc.tensor.transpose(out=tpr[:], in_=gr[:], identity=ident[:])
                    tss = tp.tile([128, H], dt.float32, tag="tss")
                    trs = tp.tile([128, H], dt.float32, tag="trs")
                    nc.vector.tensor_copy(out=tss[:], in_=tps[:])
                    nc.vector.tensor_copy(out=trs[:], in_=tpr[:])
                    pm = pp.tile([128, H], dt.float32, tag="pm")
                    nc.tensor.matmul(out=pm[:], lhsT=tss[:], rhs=wa_sb[:],
                                     start=True, stop=False)
                    nc.tensor.matmul(out=pm[:], lhsT=trs[:], rhs=wb_sb[:],
                                     start=False, stop=False)
                    nc.tensor.matmul(out=pm[:], lhsT=ea_sb[:, js], rhs=wc_sb[:],
                                     start=False, stop=True)
                    sg = sp.tile([128, H], dt.float32, tag="sg")
                    nc.scalar.activation(
                        out=sg[:], in_=pm[:],
                        func=mybir.ActivationFunctionType.Sigmoid)
                    nc.vector.tensor_tensor(
                        out=msg[:, j, :], in0=pm[:], in1=sg[:],
                        op=mybir.AluOpType.mult)
                    nc.vector.tensor_tensor(
                        out=tt[:, j, :], in0=msg[:, j, :], in1=w2_sb[:],
                        op=mybir.AluOpType.mult)
                nc.vector.tensor_reduce(
                    out=red[:], in_=tt[:, :, :],
                    axis=mybir.AxisListType.X, op=mybir.AluOpType.add)
                nc.scalar.activation(
                    out=gate[:], in_=red[:],
                    func=mybir.ActivationFunctionType.Sigmoid, bias=b2val)
                for j in range(ST):
                    nc.vector.tensor_tensor(
                        out=ff[:, j, :], in0=msg[:, j, :],
                        in1=gate[:, j : j + 1].to_broadcast([128, H]),
                        op=mybir.AluOpType.mult)
                for j in range(ST):
                    q0 = cl * ST + j
                    nc.gpsimd.indirect_dma_start(
                        out=out[:, :],
                        out_offset=bass.IndirectOffsetOnAxis(
                            ap=ri_sb[:, q0 : q0 + 1], axis=0),
                        in_=ff[:, j, :], in_offset=None,
                        compute_op=mybir.AluOpType.add,
                    )
    nc.compile()
    _prog_cache[key] = nc
    return nc


def _host_prep(x_send, x_rec, index, edge_attr, bn_gamma, bn_beta, bn_mean,
               bn_var, W1, b1, W2, b2):
    s = np.asarray(index[0], dtype=np.int64)
    r = np.asarray(index[1], dtype=np.int64)
    ea = np.asarray(edge_attr, dtype=np.float32)

    scale = np.asarray(bn_gamma) / np.sqrt(np.asarray(bn_var) + BN_EPS)
    shift = np.asarray(bn_beta) - np.asarray(bn_mean) * scale
    W1f = (np.asarray(W1) * scale[:, None]).astype(np.float32)
    b1f = (np.asarray(b1) + shift @ np.asarray(W1)).astype(np.float32)

    xs_f = np.asarray(x_send, dtype=np.float32)
    wa = W1f[:H]
    wb = W1f[H : 2 * H]
    wc = np.concatenate([W1f[2 * H :], b1f[None, :]], axis=0)
    w2b = np.broadcast_to(np.asarray(W2, dtype=np.float32).reshape(1, H),
                          (128, H)).copy()
    b2val = float(np.asarray(b2).reshape(-1)[0])

    in_maps = []
    for k in range(NCORES):
        m = (r // NLOC) == k
        sk = s[m]
        rk = (r[m] - k * NLOC).astype(np.int64)
        eak = ea[m]
        n = sk.shape[0]
        assert n <= SLOTS, f"shard overflow {n}"
        xr_loc = np.zeros((NLOC + 1, H), dtype=np.float32)
        xr_loc[:NLOC] = np.asarray(x_rec[k * NLOC : (k + 1) * NLOC],
                                   dtype=np.float32)
        sidx = np.zeros((128, SLOTS // 128), dtype=np.int32)
        ridx = np.full((128, SLOTS // 128), NLOC, dtype=np.int32)
        eat = np.zeros((INV + 1, SLOTS), dtype=np.float32)
        eat[INV, :] = 1.0
        # sort by receiver, spread column-major over chunks so receivers are
        # distinct within each chunk (and each 128-subtile)
        o = np.argsort(rk, kind="stable")
        sk, rk, eak = sk[o], rk[o], eak[o]
        i = np.arange(n)
        c = i % NCHUNK
        q = i // NCHUNK          # slot within chunk, < 2048
        col = c * ST + q // 128  # subtile column
        row = q % 128            # partition
        sidx[row, col] = sk.astype(np.int32)
        ridx[row, col] = rk.astype(np.int32)
        eat[:INV, c * CHUNK + q] = eak.T
        in_maps.append({
            "xs": xs_f, "xr": xr_loc, "sidx": sidx, "ridx": ridx,
            "eat": eat, "wa": wa, "wb": wb, "wc": wc, "w2b": w2b,
        })
    return in_maps, b2val


def kernel(**inputs) -> np.ndarray:
    in_maps, b2val = _host_prep(**inputs)
    nc = _build(b2val)
    res = run_bass_kernel_spmd(nc, in_maps, core_ids=list(range(NCORES)))
    return np.concatenate(
        [res.results[k]["out"][:NLOC] for k in range(NCORES)], axis=0
    ).astype(np.float32)
